# revision 43
# baseline (speedup 1.0000x reference)
"""DigitCapsule dynamic-routing kernel for 8 Trainium2 NeuronCores.

Key restructuring: u_hat (B,R,D,O) = 188 MB is NEVER materialized.
  s[b,(d,o)]  = sum_{(r,i)} (c[r,d]*W[r,d,o,i]) * u[b,r,i]      (matmul over (r,i))
  b_upd[r,d]  = sum_{i,o} W[r,d,o,i] * G[(r,i),(d,o)],
  G[(r,i),(d,o)] = sum_b u[b,(r,i)] * v[b,(d,o)]                 (matmul over b)

Sharding: route nodes R=1152 are split 144/core across 8 cores.  Softmax
(over d) and the b-logit update are then fully local; the only collective
is one 160 KB fp32 AllReduce of the partial s per routing iteration (the
3rd iteration ships partials; the host gather sums and squashes).

v2 (fp16 compute): all matmul operands are fp16 (PE runs 1 cycle/row vs 4
for fp32), inputs are loaded as fp16 (halves HBM traffic), accumulation
stays fp32 in PSUM, and the AllReduce payload stays fp32/160KB (sub-256KB
collectives are latency-bound, so shrinking it would only game the model).
The (i,o) double contraction of the b-logit update runs as accumulating
J-matmuls (one per o, strided view of Ht = W.*G), folding the o-sum into
the i-sum/broadcast so the DVE does a single W*G elementwise pass; the
update is split at the t=6 group boundary so the softmax/CW/mm1 stream of
t0-5 overlaps the tail of the agreement math.  Iteration 1's exp reads
bd*g straight out of PSUM (Act scale slot), dropping the blog write off
the critical chain.  Idle-window matmul chains - gated to start only
after each round's mm1 - keep the PE's HAM clock at 2.4 GHz across the
collectives without stealing issue slots from CW-gated real matmuls.

Layouts on device (per core):
  u_nat [128,2,1152] f16 : u[b,(r,i)] with b = h*128+p
  uT    [128,9,256]  f16 : u[(r,i),b] with (r,i) = t*128+p
  Wp    [128,9,160]  f16 : W[(r,i),(d,o)]  ((r,i)=t*128+p, f=d*16+o)
  Jm    [128,128]    f16 : block-diag ones (16 blocks of 8x8) - sums/
                           broadcasts over the i sub-axis via the PE
The device tracks s_dev = A*s_true (A=10 for iteration 0, which skips the
softmax and feeds W straight to mm1; A=1 after) and corrects inside
squash: the scalar g = sqrt(T)/(A^2+T), T = sum(s_dev^2), folds into the
W*G multiply of the b-logit update.
"""

import numpy as np

import concourse.bass as bass
import concourse.mybir as mybir
import concourse.tile as tile
from concourse.bass_utils import run_bass_kernel_spmd
from concourse.tile import add_dep_helper

N_CORES = 8
B, R, D, O, I_CH = 256, 1152, 10, 16, 8
RL = R // N_CORES           # 144 route nodes per core
KRI = RL * I_CH             # 1152 = (r,i) contraction length per core
NT = KRI // 128             # 9 partition tiles of (r,i)
DO = D * O                  # 160
NB = B // 128               # 2 batch halves
N_ITER = 3

f32 = mybir.dt.float32
f16 = mybir.dt.float16
ALU = mybir.AluOpType
AF = mybir.ActivationFunctionType
AX = mybir.AxisListType

_ws_ctr = [0]


def _split_excess_waits(nc, max_waits=1):
    """Walrus in this container only lowers one sync-wait per instruction.
    Hoist excess waits onto NOPs inserted before the instruction on the
    same engine (same-order execution => identical semantics)."""
    n_split = 0
    for f in nc.m.functions:
        for bb in f.blocks:
            out = []
            changed = False
            for ins in bb.instructions:
                si = ins.sync_info
                waits = list(si.on_wait) if (si is not None and si.on_wait) else []
                if len(waits) > max_waits:
                    changed = True
                    n_split += 1
                    head, rest = waits[:-max_waits], waits[-max_waits:]
                    while head:
                        chunk, head = head[:max_waits], head[max_waits:]
                        _ws_ctr[0] += 1
                        nop = mybir.InstNoOp(name=f"I-ws{_ws_ctr[0]}")
                        nop.engine = ins.engine
                        nop.sync_info = mybir.SyncInfo(on_wait=chunk, on_update=[])
                        out.append(nop)
                    ins.sync_info = mybir.SyncInfo(
                        on_wait=rest,
                        on_update=list(si.on_update) if si.on_update else [],
                    )
                out.append(ins)
            if changed:
                bb.instructions = out
    return n_split


def _build_nc(warm0=5, warm_ar=(70, 62), warm_gap=5):
    nc = bass.Bass(
        "TRN2", target_bir_lowering=False, debug=False, num_devices=N_CORES
    )
    u_nat_d = nc.dram_tensor("u_nat", [NB, 128, KRI], f16, kind="ExternalInput")
    uT_d = nc.dram_tensor("uT", [128, NT, B], f16, kind="ExternalInput")
    Wp_d = nc.dram_tensor("Wp", [128, NT, DO], f16, kind="ExternalInput")
    Jm_d = nc.dram_tensor("Jm", [128, 128], f16, kind="ExternalInput")
    v_out_d = nc.dram_tensor("v_out", [NB, 128, DO], f32, kind="ExternalOutput")

    rg = [list(range(N_CORES))]
    groups = [(0, 3), (3, 6), (6, 9)]

    with tile.TileContext(nc) as tc:
        with (
            tc.tile_pool(name="persist", bufs=1) as pp_,
            tc.tile_pool(name="iter", bufs=2) as ip_,
            tc.tile_pool(name="small", bufs=2) as sp_,
            tc.tile_pool(name="dram", bufs=2, space="DRAM") as dp_,
            tc.tile_pool(name="ps_s", bufs=1, space="PSUM") as ps_s,
            tc.tile_pool(name="ps_g", bufs=3, space="PSUM") as ps_g,
            tc.tile_pool(name="ps_bd", bufs=1, space="PSUM") as ps_bd,
            tc.tile_pool(name="ps_t", bufs=1, space="PSUM") as ps_t,
        ):
            # ---- persistent tensors ----
            u_nat = pp_.tile([128, NB, KRI], f16)
            uT = pp_.tile([128, NT, B], f16)
            Wp = pp_.tile([128, NT, DO], f16)
            J = pp_.tile([128, 128], f16)
            ones = pp_.tile([128, 512], f16)
            ones32 = pp_.tile([128, 128], f32)
            blog = pp_.tile([128, NT, D], f32)

            # uT+Wp gate mm1 of iteration 0 -> loaded first on two HWDGE
            # queues, in the order mm1 consumes them; u_nat/J ride the Pool
            # SWDGE queue and are deferred past the AR-input DMAs.
            nc.sync.dma_start(Wp[:, 0:6, :], Wp_d[:, 0:6, :])
            nc.scalar.dma_start(uT[:, 0:6, :], uT_d[:, 0:6, :])
            nc.sync.dma_start(Wp[:, 6:9, :], Wp_d[:, 6:9, :])
            nc.scalar.dma_start(uT[:, 6:9, :], uT_d[:, 6:9, :])
            nc.vector.memset(ones[:], 1.0)
            nc.gpsimd.memset(ones32[:], 1.0)
            # Warm the PE HAM clock while the uT/Wp DMAs are in flight so
            # iteration 0's mm1 doesn't start at 1/4 clock.
            if warm0:
                pw_ps = ps_t.tile([128, 512], f32, name="pw", tag="wm")
                for k in range(warm0):
                    nc.tensor.matmul(
                        pw_ps[:], ones[:, 0:128], ones[:], start=True, stop=True
                    )
            deferred_loads = [False]

            def _emit_deferred_loads(anchor):
                # u_nat/J are only needed by mm2 (after AR0 returns).  Gate
                # them on iteration 0's last inb DMA so the shared DMA
                # engines finish the uT/Wp/inb transfers first; they ride
                # the Act HWDGE queue, which idles through AR0 (the Pool
                # queue carries the collective and must stay clear).
                if deferred_loads[0]:
                    return
                deferred_loads[0] = True
                for h in range(NB):
                    d = nc.scalar.dma_start(u_nat[:, h, :], u_nat_d[h])
                    add_dep_helper(d.ins, anchor.ins, sync=True,
                                   reason="defer u_nat load past uT/Wp/inb")
                dj = nc.scalar.dma_start(J[:], Jm_d[:])
                add_dep_helper(dj.ins, anchor.ins, sync=True,
                               reason="defer J load past uT/Wp/inb")

            sf32 = None
            for it in range(N_ITER):
                last = it == N_ITER - 1
                if it == 0:
                    # b==0 => c uniform: feed W directly, fold the 1/10 into
                    # the squash constants (s_dev = 10 * s_true => A^2=100).
                    CW = Wp
                else:
                    # ---- agreement + b-logit update from previous AR ----
                    sf16 = ip_.tile([128, NB, DO], f16, name=f"sf16_{it}", tag="sf16")
                    for h in range(NB):
                        nc.vector.tensor_copy(sf16[:, h, :], sf32[:, h, :])
                    # squash scalars: T = sum(s_dev^2) over the full batch,
                    # g = sqrt(T)/(A^2+T); s_dev = A*s_true.
                    sqscr = sp_.tile([128, NB * DO], f32, name=f"sq{it}", tag="sq")
                    ppsum = sp_.tile([128, 1], f32, name=f"pps{it}", tag="pps")
                    nc.scalar.activation(
                        sqscr[:], sf32[:].rearrange("p h f -> p (h f)"), AF.Square,
                        accum_out=ppsum[:],
                    )
                    Ht = ip_.tile([128, NT, DO], f16, name=f"ht{it}", tag="ht")
                    for gi, (lo, hi) in enumerate(groups):
                        n = hi - lo
                        G_ps = ps_g.tile(
                            [128, n, DO], f32, name=f"G{it}_{lo}", tag="G"
                        )
                        for k, t in enumerate(range(lo, hi)):
                            for h in range(NB):
                                nc.tensor.matmul(
                                    G_ps[:, k, :],
                                    u_nat[:, h, t * 128: (t + 1) * 128],
                                    sf16[:, h, :],
                                    start=(h == 0),
                                    stop=(h == NB - 1),
                                )
                        if gi == 0:
                            # T broadcast to every partition via ones-matmul;
                            # emitted after the first G group so the PE is not
                            # stalled waiting on the Act Square accumulator.
                            T_ps = ps_t.tile([128, 1], f32, name=f"T{it}", tag="wm")
                            nc.tensor.matmul(
                                T_ps[:], ones32[:], ppsum[:], start=True, stop=True
                            )
                            q = sp_.tile([128, 1], f32, name=f"q{it}", tag="q")
                            # A^2 of the iteration that PRODUCED sf (it-1)
                            nc.vector.tensor_scalar_add(
                                q[:], T_ps[:], 100.0 if it == 1 else 1.0
                            )
                            qinv = sp_.tile([128, 1], f32, name=f"qi{it}", tag="qi")
                            nc.vector.reciprocal(qinv[:], q[:])
                            rt = sp_.tile([128, 1], f32, name=f"rt{it}", tag="rt")
                            nc.scalar.activation(rt[:], T_ps[:], AF.Sqrt)
                            g = sp_.tile([128, 1], f32, name=f"g{it}", tag="g")
                            nc.vector.tensor_tensor(g[:], rt[:], qinv[:], op=ALU.mult)
                        # Ht = G .* W  (fp16 out; the squash scalar g is
                        # applied later on the 10x smaller blog update, so
                        # this pass is not gated on the squash chain)
                        nc.vector.tensor_tensor(
                            Ht[:, lo:hi, :], G_ps[:], Wp[:, lo:hi, :], op=ALU.mult
                        )
                    # b_upd[(r,i),(t,d)] = sum_i sum_o Ht: accumulating
                    # J-matmuls, one per o over a strided view of Ht -- the
                    # o-sum rides the PE instead of a DVE reduce.  The
                    # update is split at t=6 (the Ht group boundary) so the
                    # softmax/CW of t0-5 overlaps group 2's Ht/bd work.
                    bd_ps = ps_bd.tile([128, NT * D], f32, name=f"bd{it}", tag="bd")
                    Htv = Ht[:].rearrange("p t (d o) -> p (t d) o", d=D, o=O)
                    e = ip_.tile([128, NT, D], f32, name=f"e{it}", tag="e")
                    den = ip_.tile([128, NT], f32, name=f"den{it}", tag="den")
                    recip = ip_.tile([128, NT], f32, name=f"rc{it}", tag="rc")
                    CW = ip_.tile([128, NT, DO], f16, name=f"cw{it}", tag="cw")
                    for tl, th in ((0, 6), (6, 9)):
                        cl, ch = tl * D, th * D
                        for o in range(O):
                            nc.tensor.matmul(
                                bd_ps[:, cl:ch], J[:], Htv[:, cl:ch, o],
                                start=(o == 0), stop=(o == O - 1),
                            )
                        bd_v = bd_ps[:, cl:ch].rearrange(
                            "p (t d) -> p t d", t=th - tl, d=D
                        )
                        # ---- softmax over d (logits replicated over the i
                        # sub-axis; o-broadcast happens inside the CW stt) ----
                        if it == 1:
                            # blog starts at 0: exp reads bd*g straight from
                            # PSUM (scale=g) and the blog write drops off the
                            # critical chain onto the DVE
                            nc.scalar.activation(
                                e[:, tl:th, :],
                                bd_ps[:, cl:ch].rearrange(
                                    "p (t d) -> p t d", t=th - tl, d=D
                                ),
                                AF.Exp, scale=g[:, 0:1],
                            )
                            nc.vector.tensor_scalar_mul(
                                blog[:, tl:th, :], bd_v, g[:, 0:1]
                            )
                        else:
                            nc.vector.scalar_tensor_tensor(
                                blog[:, tl:th, :], bd_v, g[:, 0:1],
                                blog[:, tl:th, :],
                                op0=ALU.mult, op1=ALU.add,
                            )
                            nc.scalar.activation(
                                e[:, tl:th, :], blog[:, tl:th, :], AF.Exp
                            )
                        nc.vector.tensor_reduce(
                            den[:, tl:th],
                            e[:, tl:th, :],
                            axis=AX.X, op=ALU.add,
                        )
                        nc.vector.reciprocal(recip[:, tl:th], den[:, tl:th])
                        for t in range(tl, th):
                            nc.vector.scalar_tensor_tensor(
                                CW[:, t, :].rearrange("p (d o) -> p d o", d=D, o=O),
                                Wp[:, t, :].rearrange("p (d o) -> p d o", d=D, o=O),
                                recip[:, t: t + 1],
                                e[:, t, :].unsqueeze(2).broadcast_to([128, D, O]),
                                op0=ALU.mult, op1=ALU.mult,
                            )
                    # a few warm matmuls bridge the PE over the bd->CW
                    # handoff so mm1 starts at full clock
                    if warm_gap:
                        wg_ps = ps_t.tile([128, 512], f32, name=f"wg{it}", tag="wm")
                        for k in range(warm_gap):
                            nc.tensor.matmul(
                                wg_ps[:], ones[:, 0:128], ones[:],
                                start=True, stop=True,
                            )
                # ---- mm1: s_dev[b,(d,o)] = sum_(r,i) uT.T @ CW ----
                # h-outer so half 0's drain + bounce DMA overlap half 1.
                s_sb = ip_.tile([128, NB, DO], f32, name=f"s{it}", tag="s")
                inb = outb = None
                if not last:
                    inb = dp_.tile(
                        [NB, 128, DO], f32, name=f"inb{it}", tag="inb"
                    )
                    outb = dp_.tile(
                        [NB, 128, DO], f32, name=f"outb{it}", tag="outb",
                        addr_space="Shared",
                    )
                last_inb = None
                h_last_mm = None
                for h in range(NB):
                    s_ps = ps_s.tile(
                        [128, DO], f32, name=f"sps{it}_{h}", tag=f"sps{h}"
                    )
                    for t in range(NT):
                        mm = nc.tensor.matmul(
                            s_ps[:],
                            uT[:, t, h * 128: (h + 1) * 128],
                            CW[:, t, :],
                            start=(t == 0),
                            stop=(t == NT - 1),
                        )
                        h_last_mm = mm
                    # mm1 is CW-production-bound so the halves finish nearly
                    # together; drain them on two different engines
                    if h == 0:
                        nc.scalar.activation(s_sb[:, h, :], s_ps[:], AF.Copy)
                    else:
                        nc.vector.tensor_copy(s_sb[:, h, :], s_ps[:])
                if last:
                    # final iteration: no AllReduce -- ship the partial s in
                    # one DMA (both halves finish together); the host gather
                    # sums the 8 partials and applies the scalar squash
                    # (part of the unshard step).
                    nc.sync.dma_start(
                        v_out_d[:].rearrange("h p f -> p h f"), s_sb[:]
                    )
                    continue
                # one bounce DMA for both halves: saves a serialized HWDGE
                # issue slot on the critical path into the collective
                last_inb = nc.sync.dma_start(
                    inb[:].rearrange("h p f -> p h f"), s_sb[:]
                )
                # ---- AllReduce partial s over the 8 cores ----
                nc.gpsimd.collective_compute(
                    "AllReduce", ALU.add, replica_groups=rg,
                    ins=[inb.opt()], outs=[outb.opt()],
                )
                if it == 0:
                    _emit_deferred_loads(last_inb)
                # keep the PE array's HAM clock warm through the collective:
                # a chain of matmuls gated after mm1's PSUM drain.
                n_warm = warm_ar[it] if isinstance(warm_ar, tuple) else warm_ar
                if n_warm:
                    wm_ps = ps_t.tile([128, 512], f32, name=f"wm{it}", tag="wm")
                    for k in range(n_warm):
                        wmi = nc.tensor.matmul(
                            wm_ps[:], ones[:, 0:128], ones[:],
                            start=True, stop=True,
                        )
                        if k == 0:
                            # start the warm chain only once this round's
                            # mm1 stream is done, else the scheduler
                            # interleaves it with CW-gated real matmuls
                            add_dep_helper(
                                wmi.ins, h_last_mm.ins, sync=True,
                                reason="warm chain after mm1",
                            )
                sf32 = ip_.tile([128, NB, DO], f32, name=f"sf{it}", tag="sf")
                for h in range(NB):
                    nc.sync.dma_start(sf32[:, h, :], outb[h])

    _split_excess_waits(nc, 1)
    return nc


_NC_CACHE = {}


def _get_nc(warm0=5, warm_ar=(70, 62), warm_gap=5):
    key = (warm0, warm_ar, warm_gap)
    if key not in _NC_CACHE:
        _NC_CACHE[key] = _build_nc(warm0=warm0, warm_ar=warm_ar,
                                   warm_gap=warm_gap)
    return _NC_CACHE[key]


def _prep_core_inputs(u, W, c):
    r0, r1 = c * RL, (c + 1) * RL
    u2 = np.ascontiguousarray(u[:, r0:r1, :]).reshape(B, KRI)
    u_nat = np.ascontiguousarray(u2.reshape(NB, 128, KRI)).astype(np.float16)
    uT = np.ascontiguousarray(
        np.ascontiguousarray(u2.T).reshape(NT, 128, B).transpose(1, 0, 2)
    ).astype(np.float16)
    Wp2 = np.ascontiguousarray(W[0, r0:r1].transpose(0, 3, 1, 2)).reshape(KRI, DO)
    Wp = np.ascontiguousarray(
        Wp2.reshape(NT, 128, DO).transpose(1, 0, 2)
    ).astype(np.float16)
    return {"u_nat": u_nat, "uT": uT, "Wp": Wp}


def kernel(u, W, _trace=False, _warm=(5, (70, 62), 5)):
    u = np.asarray(u, dtype=np.float32)
    W = np.asarray(W, dtype=np.float32)
    assert u.shape == (B, R, I_CH) and W.shape == (1, R, D, O, I_CH)
    Jm = np.kron(np.eye(16, dtype=np.float16), np.ones((8, 8), np.float16))
    in_maps = []
    for c in range(N_CORES):
        m = _prep_core_inputs(u, W, c)
        m["Jm"] = Jm
        in_maps.append(m)
    nc = _get_nc(*_warm)
    res = run_bass_kernel_spmd(
        nc, in_maps, core_ids=list(range(N_CORES)), trace=_trace
    )
    # unshard: final-iteration s is reduction-sharded over cores; sum the
    # partials and apply the scalar squash v = s*||s||/(1+||s||^2).
    s = np.zeros((B, DO), dtype=np.float32)
    for r in res.results:
        s += np.asarray(r["v_out"], dtype=np.float32).reshape(B, DO)
    n2 = float((s * s).sum())
    v = (s * (np.sqrt(n2) / (1.0 + n2))).astype(np.float32).reshape(B, D, O)
    if _trace:
        return v, res
    return v
